# revision 2
# baseline (speedup 1.0000x reference)
"""TRN2 Bass kernel for OneLayerCNN: conv2d(4x4, stride 2, pad 2) + bias + ReLU.

Input  A_prev (64, 256, 256, 3) f32, W (4,4,3,16), b (1,1,1,16)
Output (64, 129*129*16) f32.

Data-parallel over 8 NeuronCores (8 images each). Weights-stationary design:

- The conv is blocked along the OUTPUT W dim: 17 w-blocks of S=8 outputs
  (16 full + 1 single).  For block B the input window spans 108 interleaved
  columns (row-pair interleave c = 2*(3x+ci) + rowparity), so the host ships
  one fp16 "strip" per block: [K_B, 1040] = [band-offset, (pair, img)] with
  a ones-row at K_B-1 for the bias.  K = 109 std / 97 (B=0, left pad
  dropped) / 13 (B=16, right pad dropped).  No transposes on device.
- Matmul roles are FLIPPED vs im2col: the banded WEIGHTS are the stationary
  operand [K_B, 128=(s,co)] (one LDWEIGHTS per block+tap, 34 total) and the
  activations STREAM as the moving operand (instances = (h',img) columns).
  Every streamed column is a real output: zero N-dim waste on the PE.
- PSUM [128=(s,co), 512 insts] per bank; 3 banks per block (512+512+8).
  tap0 streams insts [a,b), tap1 streams [a+8,b+8) into the SAME psum cols
  (accumulate), implementing the two row-pairs of the 4-row filter.
- Eviction = pure ReLU (bias came via the ones-row), alternating DVE/ACT,
  into per-block oacc [128, 1032] fp16; one output DMA per block
  ([17,128,1032] DRAM, 2064B/partition runs).  Host un-permutes
  (B,s,co,h',img) -> (img,h',w',co) and upcasts to f32.
- DMA queues: sync carries WP + even strips, gpsimd carries odd strips and
  all output DMAs -- keeps the Sync sequencer from serializing issue.
- PE warmup matmuls on a memset dummy tile (no DMA dependency) open the HAM
  clock gate during the initial input DMA.
- The bass kernel-semaphore range is narrowed (fewer sems declared -> the
  NEFF's fixed per-semaphore init/teardown sweep shrinks measurably).
A post-pass splits multi-sem-wait instructions (walrus accepts one sync
wait per instruction).
"""
import numpy as np
from contextlib import ExitStack

import concourse.bass as bass
import concourse.tile as tile
from concourse import mybir
from concourse.bass_utils import run_bass_kernel_spmd
from concourse.env import get_walrus_max_sem_num
import bass_rust

# ---------------- problem constants (hardcoded) ----------------
N_CORES = 8
IMG = 8              # images per core
H = 256
WID = 256
CIN = 3
F = 4
COUT = 16
HO = 129
WO = 129
S = 8                # w' outputs per full block
NB = 17              # w-blocks (16 full + 1 of 1 output)
NI = 1040            # instance columns: 130 row-pairs x 8 img
NVAL = 1032          # valid output instances: 129 h' x 8 img
N_SEMS = 48          # narrowed kernel semaphore range
N_WARM = 10          # PE warmup matmuls (HAM clock-gate opener)

DT = mybir.dt.float16
DT32 = mybir.dt.float32

BANKS = ((0, 512), (512, 1024), (1024, 1032))


def _kb1(B):
    """strip partition count for block B (incl. the ones/bias row)."""
    return 97 if B == 0 else (13 if B == 16 else 109)


def _mb(B):
    return 16 if B == 16 else 128


def _split_multi_waits(nc):
    """walrus accepts at most ONE sync wait per instruction; hoist extras
    onto NoOps inserted just before, same engine queue."""
    ctr = 0
    for f in nc.m.functions:
        for bb in f.blocks:
            insts = bb.instructions  # live list
            out = []
            changed = False
            for inst in insts:
                si = inst.sync_info
                if si is None:
                    out.append(inst)
                    continue
                waits = list(si.on_wait)
                if len(waits) > 1:
                    changed = True
                    for w in waits[:-1]:
                        ctr += 1
                        nop = mybir.InstNoOp(name=f"I-wsplit-{ctr}")
                        nop.engine = inst.engine
                        nop.sync_info = bass_rust.SyncInfo(
                            on_wait=[w], on_update=[])
                        out.append(nop)
                    inst.sync_info = bass_rust.SyncInfo(
                        on_wait=[waits[-1]], on_update=list(si.on_update))
                out.append(inst)
            if changed:
                insts[:] = out
    return nc


def _make_weights(W, b):
    """WP[r, col] fp16: cols 0:128 std_t0 | 128:256 std_t1 | 256:384 B0_t0
    | 384:512 B0_t1 | 512:528 B16_t0 | 528:544 B16_t1.

    std[r = 12s+6fw+2ci+q, 16s+co] = W[2t+q, fw, ci, co]; B0 shifts r by
    -12 (drops the left-pad taps), B16 keeps only fw<2 (right pad).  The
    tap0 variant carries bias[co] in its last row (multiplied by the
    strips' ones-row); tap1's last row is zero."""
    WP = np.zeros((128, 544), dtype=np.float32)
    bias = b.reshape(-1)

    def fill(col0, M, tap, rshift, fwmax, krows):
        for s in range(M // COUT):
            for fw in range(fwmax):
                for ci in range(CIN):
                    for q in range(2):
                        r = 12 * s + 6 * fw + 2 * ci + q - rshift
                        if 0 <= r < krows - 1:
                            WP[r, col0 + COUT * s:col0 + COUT * (s + 1)] = \
                                W[2 * tap + q, fw, ci]
        if tap == 0:
            WP[krows - 1, col0:col0 + M] = np.tile(bias, M // COUT)

    fill(0, 128, 0, 0, 4, 109)
    fill(128, 128, 1, 0, 4, 109)
    fill(256, 128, 0, 12, 4, 97)
    fill(384, 128, 1, 12, 4, 97)
    fill(512, 16, 0, 0, 2, 13)
    fill(528, 16, 1, 0, 2, 13)
    return WP.astype(np.float16)


def _make_strips(A_core):
    """Per-core input -> list of 17 strip arrays [K_B, 1040] fp16.

    G[img, p', c]: p' = pair+1 (pairs -1..128), c = 2*(3x+ci)+rowparity.
    Strip B = G[:, :, c0:c0+K-1] transposed to [K-1, (p', img)], ones row
    at K-1."""
    A16 = A_core.reshape(IMG, H, WID * CIN).astype(np.float16)
    G = np.zeros((IMG, 130, 2 * WID * CIN), dtype=np.float16)
    G[:, 1:129, 0::2] = A16[:, 0::2, :]
    G[:, 1:129, 1::2] = A16[:, 1::2, :]
    strips = []
    for B in range(NB):
        c0 = max(0, 96 * B - 12)
        K1 = _kb1(B)
        st = np.empty((K1, NI), dtype=np.float16)
        st[0:K1 - 1] = np.transpose(G[:, :, c0:c0 + K1 - 1], (2, 1, 0)
                                    ).reshape(K1 - 1, NI)
        st[K1 - 1] = 1.0
        strips.append(np.ascontiguousarray(st))
    return strips


def _build_nc():
    start = get_walrus_max_sem_num()
    orig_range = bass.get_kernel_semaphore_range
    bass.get_kernel_semaphore_range = lambda: range(start, start + N_SEMS)
    try:
        nc = bass.Bass()
    finally:
        bass.get_kernel_semaphore_range = orig_range

    a_in = [nc.declare_dram_parameter(f"A{B}", [_kb1(B), NI], DT,
                                      isOutput=False) for B in range(NB)]
    w_in = nc.declare_dram_parameter("WP", [128, 544], DT, isOutput=False)
    z_out = nc.declare_dram_parameter("Z", [NB, 128, NVAL], DT,
                                      isOutput=True)

    with tile.TileContext(nc) as tc, ExitStack() as ctx:
        wpool = ctx.enter_context(tc.tile_pool(name="w", bufs=1))
        spool = ctx.enter_context(tc.tile_pool(name="strips", bufs=1))
        opool = ctx.enter_context(tc.tile_pool(name="oacc", bufs=4))
        ppool = ctx.enter_context(
            tc.tile_pool(name="pconv", bufs=7, space="PSUM"))
        pw_pool = ctx.enter_context(
            tc.tile_pool(name="pwarm", bufs=1, space="PSUM"))

        # weights first on sync (tiny; unblocks all matmuls), then strips:
        # even B on sync, odd B on gpsimd so neither sequencer serializes
        # the whole input stream.
        wt = wpool.tile([128, 544], DT, tag="wt", name="wt")
        nc.sync.dma_start(out=wt[:], in_=w_in[:])

        # warmup dummy: memset (no DMA dep) so the PE can start opening the
        # HAM clock gate immediately.
        dummy = wpool.tile([128, 128], DT, tag="dummy", name="dummy")
        nc.gpsimd.memset(dummy[:], 0.002)

        stt = []
        for B in range(NB):
            t = spool.tile([128, NI], DT, tag=f"s{B}", name=f"s{B}")
            stt.append(t)
        for B in range(0, NB, 2):
            nc.sync.dma_start(out=stt[B][0:_kb1(B), :], in_=a_in[B][:])
        for B in range(1, NB, 2):
            nc.gpsimd.dma_start(out=stt[B][0:_kb1(B), :], in_=a_in[B][:])

        pwarm = pw_pool.tile([128, 512], DT32, tag="pwarm", name="pwarm")
        for _ in range(N_WARM):
            nc.tensor.matmul(pwarm[:, 0:128], dummy[:], dummy[:],
                             start=True, stop=True)

        def wsl(B, tap):
            K1 = _kb1(B)
            if B == 0:
                return wt[0:K1, 256 + 128 * tap:384 + 128 * tap]
            if B == 16:
                return wt[0:K1, 512 + 16 * tap:528 + 16 * tap]
            return wt[0:K1, 128 * tap:128 * (tap + 1)]

        ev = 0
        for B in range(NB):
            K1 = _kb1(B)
            M = _mb(B)
            w0, w1 = wsl(B, 0), wsl(B, 1)
            st = stt[B]
            oacc = opool.tile([128, NVAL], DT, tag="oacc")
            for (a, b_) in BANKS:
                N = b_ - a
                pc = ppool.tile([128, 512], DT32, tag="pc")
                nc.tensor.matmul(pc[0:M, 0:N], w0, st[0:K1, a:b_],
                                 start=True, stop=False)
                nc.tensor.matmul(pc[0:M, 0:N], w1, st[0:K1, a + 8:b_ + 8],
                                 start=False, stop=True)
                dst = oacc[0:M, a:b_]
                if ev % 2 == 1:
                    nc.scalar.activation(dst, pc[0:M, 0:N],
                                         mybir.ActivationFunctionType.Relu)
                else:
                    nc.vector.tensor_scalar_max(dst, pc[0:M, 0:N], 0.0)
                ev += 1
            nc.gpsimd.dma_start(out=z_out[B, 0:M, :], in_=oacc[0:M, :])

    _split_multi_waits(nc)
    return nc


_NC_CACHE = {}


def _get_nc():
    if "nc" not in _NC_CACHE:
        _NC_CACHE["nc"] = _build_nc()
    return _NC_CACHE["nc"]


def _unpermute(Z):
    """[17, 128, 1032] fp16 -> [8, 129*129*16] f32 for one core."""
    v = Z.astype(np.float32).reshape(NB, S, COUT, HO, IMG)
    v = np.transpose(v, (4, 3, 0, 1, 2)).reshape(IMG, HO, NB * S, COUT)
    return v[:, :, 0:WO, :].reshape(IMG, -1)


def kernel(A_prev, W, b, _trace=False, _dt=None):
    A_prev = np.ascontiguousarray(A_prev, dtype=np.float32)
    W = np.asarray(W, dtype=np.float32)
    b = np.asarray(b, dtype=np.float32)
    WP = _make_weights(W, b)

    nc = _get_nc()
    in_maps = []
    for c in range(N_CORES):
        strips = _make_strips(A_prev[c * IMG:(c + 1) * IMG])
        m = {f"A{B}": strips[B] for B in range(NB)}
        m["WP"] = WP
        in_maps.append(m)

    res = run_bass_kernel_spmd(nc, in_maps, list(range(N_CORES)),
                               trace=_trace)
    out = np.concatenate(
        [_unpermute(res.results[c]["Z"]) for c in range(N_CORES)], axis=0)
    if _trace:
        return out, res
    return out


# revision 4
# speedup vs baseline: 2.6040x; 2.6040x over previous
"""TRN2 Bass kernel for OneLayerCNN: conv2d(4x4, stride 2, pad 2) + bias + ReLU.

Input  A_prev (64, 256, 256, 3) f32, W (4,4,3,16), b (1,1,1,16)
Output (64, 129*129*16) f32.

Data-parallel over 8 NeuronCores (8 images each). Weights-stationary design:

- The conv is blocked along the OUTPUT W dim: 17 w-blocks of S=8 outputs
  (16 full + 1 single).  For block B the input window spans 108 interleaved
  columns (row-pair interleave c = 2*(3x+ci) + rowparity), so the host ships
  one fp16 "strip" per block: [K_B, 1040] = [band-offset, (pair, img)] with
  a ones-row at K_B-1 for the bias.  K = 109 std / 97 (B=0, left pad
  dropped) / 13 (B=16, right pad dropped).  No transposes on device.
- Matmul roles are FLIPPED vs im2col: the banded WEIGHTS are the stationary
  operand [K_B, 128=(s,co)] (one LDWEIGHTS per block+tap, 34 total) and the
  activations STREAM as the moving operand (instances = (h',img) columns).
  Every streamed column is a real output: zero N-dim waste on the PE.
- PSUM [128=(s,co), 512 insts] per bank; 3 banks per block (512+512+8).
  tap0 streams insts [a,b), tap1 streams [a+8,b+8) into the SAME psum cols
  (accumulate), implementing the two row-pairs of the 4-row filter.
- Eviction = pure ReLU (bias came via the ones-row), alternating DVE/ACT,
  into per-block oacc [128, 1032] fp16; one output DMA per block
  ([17,128,1032] DRAM, 2064B/partition runs).  Host un-permutes
  (B,s,co,h',img) -> (img,h',w',co) and upcasts to f32.
- DMA queues: sync carries WP + even strips, gpsimd carries odd strips and
  all output DMAs -- keeps the Sync sequencer from serializing issue.
- PE warmup matmuls on a memset dummy tile (no DMA dependency) open the HAM
  clock gate during the initial input DMA.
- The bass kernel-semaphore range is narrowed (fewer sems declared -> the
  NEFF's fixed per-semaphore init/teardown sweep shrinks measurably).
A post-pass splits multi-sem-wait instructions (walrus accepts one sync
wait per instruction).
"""
import numpy as np
from contextlib import ExitStack

import concourse.bass as bass
import concourse.tile as tile
from concourse import mybir
from concourse.bass_utils import run_bass_kernel_spmd
from concourse.env import get_walrus_max_sem_num
import bass_rust

# ---------------- problem constants (hardcoded) ----------------
N_CORES = 8
IMG = 8              # images per core
H = 256
WID = 256
CIN = 3
F = 4
COUT = 16
HO = 129
WO = 129
S = 8                # w' outputs per full block
NB = 17              # w-blocks (16 full + 1 of 1 output)
NI = 1040            # instance columns: 130 row-pairs x 8 img
NVAL = 1032          # valid output instances: 129 h' x 8 img
N_SEMS = 48          # narrowed kernel semaphore range
N_WARM = 10          # PE warmup matmuls (HAM clock-gate opener)

DT = mybir.dt.float16
DT32 = mybir.dt.float32

BANKS = ((0, 512), (512, 1024), (1024, 1032))


def _kb1(B):
    """strip partition count for block B (incl. the ones/bias row)."""
    return 97 if B == 0 else (13 if B == 16 else 109)


def _kpad(B):
    """strip DMA partition count: multiple of 16 so the HWDGE spreads the
    transfer across all 16 DMA engines (observed: 109-partition DMAs land
    on a single engine, 128-partition ones fan out)."""
    return 16 if B == 16 else 112


def _mb(B):
    return 16 if B == 16 else 128


def _split_multi_waits(nc):
    """walrus accepts at most ONE sync wait per instruction; hoist extras
    onto NoOps inserted just before, same engine queue."""
    ctr = 0
    for f in nc.m.functions:
        for bb in f.blocks:
            insts = bb.instructions  # live list
            out = []
            changed = False
            for inst in insts:
                si = inst.sync_info
                if si is None:
                    out.append(inst)
                    continue
                waits = list(si.on_wait)
                if len(waits) > 1:
                    changed = True
                    for w in waits[:-1]:
                        ctr += 1
                        nop = mybir.InstNoOp(name=f"I-wsplit-{ctr}")
                        nop.engine = inst.engine
                        nop.sync_info = bass_rust.SyncInfo(
                            on_wait=[w], on_update=[])
                        out.append(nop)
                    inst.sync_info = bass_rust.SyncInfo(
                        on_wait=[waits[-1]], on_update=list(si.on_update))
                out.append(inst)
            if changed:
                insts[:] = out
    return nc


def _make_weights(W, b):
    """WP[r, col] fp16: cols 0:128 std_t0 | 128:256 std_t1 | 256:384 B0_t0
    | 384:512 B0_t1 | 512:528 B16_t0 | 528:544 B16_t1.

    std[r = 12s+6fw+2ci+q, 16s+co] = W[2t+q, fw, ci, co]; B0 shifts r by
    -12 (drops the left-pad taps), B16 keeps only fw<2 (right pad).  The
    tap0 variant carries bias[co] in its last row (multiplied by the
    strips' ones-row); tap1's last row is zero."""
    WP = np.zeros((128, 544), dtype=np.float32)
    bias = b.reshape(-1)

    def fill(col0, M, tap, rshift, fwmax, krows):
        for s in range(M // COUT):
            for fw in range(fwmax):
                for ci in range(CIN):
                    for q in range(2):
                        r = 12 * s + 6 * fw + 2 * ci + q - rshift
                        if 0 <= r < krows - 1:
                            WP[r, col0 + COUT * s:col0 + COUT * (s + 1)] = \
                                W[2 * tap + q, fw, ci]
        if tap == 0:
            WP[krows - 1, col0:col0 + M] = np.tile(bias, M // COUT)

    fill(0, 128, 0, 0, 4, 109)
    fill(128, 128, 1, 0, 4, 109)
    fill(256, 128, 0, 12, 4, 97)
    fill(384, 128, 1, 12, 4, 97)
    fill(512, 16, 0, 0, 2, 13)
    fill(528, 16, 1, 0, 2, 13)
    return WP.astype(np.float16)


def _make_strips(A_core):
    """Per-core input -> list of 17 strip arrays [K_B, 1040] fp16.

    G[img, p', c]: p' = pair+1 (pairs -1..128), c = 2*(3x+ci)+rowparity.
    Strip B = G[:, :, c0:c0+K-1] transposed to [K-1, (p', img)], ones row
    at K-1."""
    A16 = A_core.reshape(IMG, H, WID * CIN).astype(np.float16)
    G = np.zeros((IMG, 130, 2 * WID * CIN), dtype=np.float16)
    G[:, 1:129, 0::2] = A16[:, 0::2, :]
    G[:, 1:129, 1::2] = A16[:, 1::2, :]
    strips = []
    for B in range(NB):
        c0 = max(0, 96 * B - 12)
        K1 = _kb1(B)
        st = np.zeros((_kpad(B), NI), dtype=np.float16)
        st[0:K1 - 1] = np.transpose(G[:, :, c0:c0 + K1 - 1], (2, 1, 0)
                                    ).reshape(K1 - 1, NI)
        st[K1 - 1] = 1.0
        strips.append(np.ascontiguousarray(st))
    return strips


def _build_nc():
    start = get_walrus_max_sem_num()
    orig_range = bass.get_kernel_semaphore_range
    bass.get_kernel_semaphore_range = lambda: range(start, start + N_SEMS)
    try:
        nc = bass.Bass()
    finally:
        bass.get_kernel_semaphore_range = orig_range

    a_in = [nc.declare_dram_parameter(f"A{B}", [_kpad(B), NI], DT,
                                      isOutput=False) for B in range(NB)]
    w_in = nc.declare_dram_parameter("WP", [128, 544], DT, isOutput=False)
    z_out = nc.declare_dram_parameter("Z", [NB, 128, NVAL], DT,
                                      isOutput=True)

    with tile.TileContext(nc) as tc, ExitStack() as ctx:
        wpool = ctx.enter_context(tc.tile_pool(name="w", bufs=1))
        spool = ctx.enter_context(tc.tile_pool(name="strips", bufs=1))
        opool = ctx.enter_context(tc.tile_pool(name="oacc", bufs=4))
        ppool = ctx.enter_context(
            tc.tile_pool(name="pconv", bufs=7, space="PSUM"))
        pw_pool = ctx.enter_context(
            tc.tile_pool(name="pwarm", bufs=1, space="PSUM"))

        # weights first on sync (tiny; unblocks all matmuls), then strips:
        # even B on sync, odd B on gpsimd so neither sequencer serializes
        # the whole input stream.
        wt = wpool.tile([128, 544], DT, tag="wt", name="wt")
        nc.sync.dma_start(out=wt[:], in_=w_in[:])

        # warmup dummy: memset (no DMA dep) so the PE can start opening the
        # HAM clock gate immediately.
        dummy = wpool.tile([128, 128], DT, tag="dummy", name="dummy")
        nc.gpsimd.memset(dummy[:], 0.002)

        stt = []
        for B in range(NB):
            t = spool.tile([128, NI], DT, tag=f"s{B}", name=f"s{B}")
            stt.append(t)
        for B in range(0, NB, 2):
            nc.sync.dma_start(out=stt[B][0:_kpad(B), :], in_=a_in[B][:])
        for B in range(1, NB, 2):
            nc.gpsimd.dma_start(out=stt[B][0:_kpad(B), :], in_=a_in[B][:])

        pwarm = pw_pool.tile([128, 512], DT32, tag="pwarm", name="pwarm")
        for _ in range(N_WARM):
            nc.tensor.matmul(pwarm[:, 0:128], dummy[:], dummy[:],
                             start=True, stop=True)

        def wsl(B, tap):
            K1 = _kb1(B)
            if B == 0:
                return wt[0:K1, 256 + 128 * tap:384 + 128 * tap]
            if B == 16:
                return wt[0:K1, 512 + 16 * tap:528 + 16 * tap]
            return wt[0:K1, 128 * tap:128 * (tap + 1)]

        ev = 0
        for B in range(NB):
            K1 = _kb1(B)
            M = _mb(B)
            w0, w1 = wsl(B, 0), wsl(B, 1)
            st = stt[B]
            oacc = opool.tile([128, NVAL], DT, tag="oacc")
            for (a, b_) in BANKS:
                N = b_ - a
                pc = ppool.tile([128, 512], DT32, tag="pc")
                nc.tensor.matmul(pc[0:M, 0:N], w0, st[0:K1, a:b_],
                                 start=True, stop=False)
                nc.tensor.matmul(pc[0:M, 0:N], w1, st[0:K1, a + 8:b_ + 8],
                                 start=False, stop=True)
                dst = oacc[0:M, a:b_]
                if ev % 2 == 1:
                    nc.scalar.activation(dst, pc[0:M, 0:N],
                                         mybir.ActivationFunctionType.Relu)
                else:
                    nc.vector.tensor_scalar_max(dst, pc[0:M, 0:N], 0.0)
                ev += 1
            nc.gpsimd.dma_start(out=z_out[B, 0:M, :], in_=oacc[0:M, :])

    _split_multi_waits(nc)
    return nc


_NC_CACHE = {}


def _get_nc():
    if "nc" not in _NC_CACHE:
        _NC_CACHE["nc"] = _build_nc()
    return _NC_CACHE["nc"]


def _unpermute(Z):
    """[17, 128, 1032] fp16 -> [8, 129*129*16] f32 for one core."""
    v = Z.astype(np.float32).reshape(NB, S, COUT, HO, IMG)
    v = np.transpose(v, (4, 3, 0, 1, 2)).reshape(IMG, HO, NB * S, COUT)
    return v[:, :, 0:WO, :].reshape(IMG, -1)


def kernel(A_prev, W, b, _trace=False, _dt=None):
    A_prev = np.ascontiguousarray(A_prev, dtype=np.float32)
    W = np.asarray(W, dtype=np.float32)
    b = np.asarray(b, dtype=np.float32)
    WP = _make_weights(W, b)

    nc = _get_nc()
    in_maps = []
    for c in range(N_CORES):
        strips = _make_strips(A_prev[c * IMG:(c + 1) * IMG])
        m = {f"A{B}": strips[B] for B in range(NB)}
        m["WP"] = WP
        in_maps.append(m)

    res = run_bass_kernel_spmd(nc, in_maps, list(range(N_CORES)),
                               trace=_trace)
    out = np.concatenate(
        [_unpermute(res.results[c]["Z"]) for c in range(N_CORES)], axis=0)
    if _trace:
        return out, res
    return out
